# revision 27
# baseline (speedup 1.0000x reference)
"""Cross-attention kernel for Trainium2 (Bass/Tile), 8-core data-parallel over batch.

Problem (per batch element b, all fp32):
    q = wq @ f1 + bq            # [32, 4096]
    k = wk @ f2 + bk            # [32, 4096]
    v = wv @ f3 + bv            # [256, 4096]
    A = softmax(q^T k, axis=m)  # [4096, 4096]   (n = query pixel, m = key pixel)
    out[c, n] = sum_m v[c, m] * A[n, m]          # [256, 4096]

Kernel strategy (flash-style, no HBM attention slab), v3:
  - One batch element per NeuronCore (B=8, 8 cores).
  - S^T tiles (m on partitions) so exp(S^T) feeds the O matmul as lhsT with
    zero transposes in the attention inner loop.
  - q/k are projected with 4x-replicated weights so q[c,n]/k[c,m] live in all
    four 32-partition groups; the K=32 S^T matmuls are then issued 4 at a time
    to distinct PE row-groups via tile_position (near-4x concurrency).
  - Everything bf16 (features, weights, q/k, exp(S), v^T): halves feature DMA
    (6.3MB/core), gives LDWEIGHTS fast-weight-load, keeps matmuls at
    1 cycle/row.  PSUM accumulation stays fp32.
  - Software pipeline: S^T+exp of block b+1 are emitted interleaved with the
    O-accumulation matmuls of block b, so the Scalar engine's exp (~2us per
    4-tile group, ~126us total) hides under PE work and the PE never idles
    long enough for HAM to re-throttle the clock.
  - A dozen fp32 identity matmuls at t~8us warm the HAM clock gate before the
    first real projection (cold PE runs at 1.2GHz vs 2.4GHz warm).
  - Phase 1 is ordered so the exp(block 0) Scalar-engine chain (the real
    phase-1 critical path) starts as early as possible: f1 chunk 0, then f2
    chunks (k projection + S^T(0,g) + exp trailing each), with f3/v-proj
    interleaved into the exp-paced slack.
  - Softmax denominators come free from a ones-column appended to v^T
    (CA=258 columns: 256 + ones + pad).  bv added at the very end.
"""

import numpy as np
import ml_dtypes
from contextlib import ExitStack

import concourse.bass as bass
import concourse.bacc as bacc
import concourse.tile as tile
from concourse import mybir
from concourse.bass_utils import run_bass_kernel_spmd
from concourse.masks import make_identity

F32 = mybir.dt.float32
BF16 = mybir.dt.bfloat16

B, C, H, W = 8, 256, 64, 64
HW = H * W                     # 4096
CQK = C // 8                   # 32
NB = 512                       # query-pixel block (free dim of S^T matmuls)
NBLK = HW // NB                # 8
MT = 128                       # key-pixel tile (partition dim of S^T)
NMT = HW // MT                 # 32
GS = 4                         # S^T matmuls packed per PE row-group volley
NG = NMT // GS                 # 8 packed groups per block
CH = C // 128                  # 2 channel halves
QCH = 512                      # projection chunk
NQC = HW // QCH                # 8
CA = C + 2                     # v_aug columns (ones + pad)

_CACHED_NC = None


def build_nc():
    nc = bacc.Bacc("TRN2")

    f1_d = nc.dram_tensor("f1", [128, CH, HW], BF16, kind="ExternalInput")
    f2_d = nc.dram_tensor("f2", [128, CH, HW], BF16, kind="ExternalInput")
    f3_d = nc.dram_tensor("f3", [128, CH, HW], BF16, kind="ExternalInput")
    wq4_d = nc.dram_tensor("wq4", [128, CH, 128], BF16, kind="ExternalInput")
    wk4_d = nc.dram_tensor("wk4", [128, CH, 128], BF16, kind="ExternalInput")
    wvT_d = nc.dram_tensor("wvT", [128, CH, C], BF16, kind="ExternalInput")
    bq4_d = nc.dram_tensor("bq4", [128, 1], F32, kind="ExternalInput")
    bvr_d = nc.dram_tensor("bvr", [1, C], BF16, kind="ExternalInput")
    out_d = nc.dram_tensor("out", [128, CH, HW], BF16, kind="ExternalOutput")

    with tile.TileContext(nc) as tc, ExitStack() as octx:
        const = octx.enter_context(tc.tile_pool(name="const", bufs=1))
        persist = octx.enter_context(tc.tile_pool(name="persist", bufs=1))
        fpool = octx.enter_context(tc.tile_pool(name="fpool", bufs=3))
        espool = octx.enter_context(tc.tile_pool(name="es", bufs=16))
        pp = octx.enter_context(tc.tile_pool(name="pp", bufs=1, space="PSUM"))
        opool = octx.enter_context(tc.tile_pool(name="outp", bufs=4))
        rpool = octx.enter_context(tc.tile_pool(name="rp", bufs=4))

        ident = const.tile([128, 128], F32)
        make_identity(nc, ident)
        wq4_sb = const.tile([128, CH, 128], BF16)
        wk4_sb = const.tile([128, CH, 128], BF16)
        wv_sb = const.tile([128, CH, C], BF16)
        bq4_sb = const.tile([128, 1], F32)
        bvr_sb = const.tile([1, C], BF16)
        ones1_sb = const.tile([1, 128], BF16)
        nc.vector.memset(ones1_sb, 1.0)
        # DMA issue order matters: the Sync queue issues one dma_start every
        # ~0.6us, so the tensors needed by the first matmuls go first (wv/bvr
        # are issued inside the f2 loop, after f2 chunk 0).
        nc.sync.dma_start(out=wk4_sb, in_=wk4_d[:])
        nc.sync.dma_start(out=wq4_sb, in_=wq4_d[:])

        # warm the HAM clock gate with ~5us of throwaway fp32 matmuls while
        # the first DMAs are in flight (cold PE = half clock; the activity
        # monitor needs ~3.4us of sustained matmul work inside one of its
        # free-running windows to unthrottle, and the warmup must also bridge
        # the PE to the arrival of the first feature chunk so no >3.4us idle
        # gap re-throttles it)
        for _ in range(12):
            ps_w = pp.tile([128, MT], F32, tag="tt", bufs=2, name="ps_w")
            nc.tensor.matmul(ps_w, lhsT=ident, rhs=ident, start=True, stop=True)

        # persistent products of phase 1 (q/k replicated across the 4
        # partition groups by construction of the replicated weights)
        q_sb = persist.tile([128, HW], BF16)
        k_sb = persist.tile([128, HW], BF16)
        vT_sb = persist.tile([128, NMT, CA], BF16)  # [128, 32, 258]
        ones_sb = const.tile([128, NMT, 2], F32)
        nc.vector.memset(ones_sb[:, :, 0:1], 1.0)
        nc.vector.memset(ones_sb[:, :, 1:2], 0.0)
        nc.vector.tensor_copy(out=vT_sb[:, :, C:CA], in_=ones_sb)

        es_map = {}

        def s_and_exp(b, g):
            """Emit 4 row-group-packed S^T matmuls (m-tiles 4g..4g+3 of query
            block b) + one exp activation over the 4-bank PSUM group."""
            nsl = slice(b * NB, (b + 1) * NB)
            ps_s = pp.tile([128, GS, NB], F32, tag="s", bufs=1, name="ps_s")
            for i in range(GS):
                u = g * GS + i
                nc.tensor.matmul(
                    ps_s[:, i, :],
                    lhsT=k_sb[32 * i : 32 * i + 32, u * MT : (u + 1) * MT],
                    rhs=q_sb[32 * i : 32 * i + 32, nsl],
                    start=True, stop=True,
                    tile_position=(32 * i, 0),
                )
            es_g = espool.tile([128, GS, NB], BF16, tag="es", bufs=16, name="es_g")
            nc.scalar.activation(
                out=es_g, in_=ps_s, func=mybir.ActivationFunctionType.Exp
            )
            es_map[(b, g)] = es_g

        f1_tiles = {}

        def f1_fetch(c):
            fch = fpool.tile([128, CH, QCH], BF16, tag="f1", bufs=3, name="f1ch")
            nc.sync.dma_start(out=fch, in_=f1_d[:, :, c * QCH : (c + 1) * QCH])
            f1_tiles[c] = fch

        def q_proj(c):
            sl = slice(c * QCH, (c + 1) * QCH)
            ps_q = pp.tile([128, QCH], F32, tag="tt", bufs=2, name="ps_q")
            nc.tensor.matmul(
                ps_q, lhsT=wq4_sb[:, 0, :], rhs=f1_tiles[c][:, 0, :],
                start=True, stop=False,
            )
            nc.tensor.matmul(
                ps_q, lhsT=wq4_sb[:, 1, :], rhs=f1_tiles[c][:, 1, :],
                start=False, stop=True,
            )
            nc.vector.tensor_scalar_add(out=q_sb[:, sl], in0=ps_q, scalar1=bq4_sb)
            del f1_tiles[c]

        def v_proj(j):
            """v^T tiles for chunk j, with bv folded in via a rank-1 matmul
            (v' = v + bv makes the final output (sum es*v')/denom = out
            directly, since softmax rows sum to 1)."""
            fch3 = fpool.tile([128, CH, QCH], BF16, tag="f3", bufs=3, name="f3ch")
            nc.sync.dma_start(out=fch3, in_=f3_d[:, :, j * QCH : (j + 1) * QCH])
            for i2 in range(2):
                # acc tag: idle during phase 1, so v-projection PSUM lives
                # there instead of contending with the k/q chain on "tt"
                ps_v = pp.tile([128, 2, C], F32, tag="acc", bufs=2, name="ps_v")
                for i in range(2):
                    isl = slice((i2 * 2 + i) * MT, (i2 * 2 + i + 1) * MT)
                    nc.tensor.matmul(
                        ps_v[:, i, :], lhsT=fch3[:, 0, isl], rhs=wv_sb[:, 0, :],
                        start=True, stop=False,
                    )
                    nc.tensor.matmul(
                        ps_v[:, i, :], lhsT=fch3[:, 1, isl], rhs=wv_sb[:, 1, :],
                        start=False, stop=False,
                    )
                    nc.tensor.matmul(
                        ps_v[:, i, :], lhsT=ones1_sb, rhs=bvr_sb,
                        start=False, stop=True,
                    )
                u = j * 4 + i2 * 2
                nc.vector.tensor_copy(out=vT_sb[:, u : u + 2, 0:C], in_=ps_v)

        # ---- phase 1 ----
        # f1 chunk 0 (for q chunk 0), then f2 chunks with k-projection +
        # S^T(0,g) + exp trailing each (k chunk g holds exactly m-tiles
        # 4g..4g+3 = S-group g); f3/v-projection fills the exp-paced slack.
        f1_fetch(0)
        nc.sync.dma_start(out=bq4_sb, in_=bq4_d[:])
        q_proj(0)
        for g in range(NQC):
            sl = slice(g * QCH, (g + 1) * QCH)
            fch2 = fpool.tile([128, CH, QCH], BF16, tag="f2", bufs=3, name="f2ch")
            nc.sync.dma_start(out=fch2, in_=f2_d[:, :, sl])
            if g == 0:
                nc.sync.dma_start(out=wv_sb, in_=wvT_d[:])
                nc.sync.dma_start(out=bvr_sb, in_=bvr_d[:])
            ps_k = pp.tile([128, QCH], F32, tag="tt", bufs=2, name="ps_k")
            nc.tensor.matmul(
                ps_k, lhsT=wk4_sb[:, 0, :], rhs=fch2[:, 0, :],
                start=True, stop=False,
            )
            nc.tensor.matmul(
                ps_k, lhsT=wk4_sb[:, 1, :], rhs=fch2[:, 1, :],
                start=False, stop=True,
            )
            # no k bias: softmax over m is invariant to the bk term, which
            # only adds n-dependent and constant offsets to q^T k
            nc.vector.tensor_copy(out=k_sb[:, sl], in_=ps_k)
            s_and_exp(0, g)
            if g >= 1:
                v_proj(g - 1)
        v_proj(NQC - 1)

        f1_fetch(1)
        f1_fetch(2)
        q_proj(1)

        # ---- phase 2: pipelined attention ----
        accs = {}
        onrms = {}

        def norm(b, j):
            """DVE part of the epilogue: 1/denominator, normalize (bv is
            already folded into v^T, so this is the final value)."""
            acc = accs.pop((b, j))
            rcp = rpool.tile([128, 1], F32, tag="r", name="rcp")
            nc.vector.reciprocal(rcp, acc[:, C : C + 1])
            onrm = rpool.tile([128, C], F32, tag="onrm", name="onrm")
            nc.vector.tensor_scalar_mul(onrm, acc[:, 0:C], rcp)
            onrms[(b, j)] = onrm

        def flush(b, j):
            """PE transposes [n, c] -> [c, n] + bf16 store (bv already folded
            into v^T, so no bias add here)."""
            onrm = onrms.pop((b, j))
            outt = opool.tile([128, CH, MT], BF16, tag="out", name="outt")
            for h in range(CH):
                ps_tt = pp.tile([128, MT], F32, tag="tt", bufs=2, name="ps_tt")
                nc.tensor.transpose(ps_tt, onrm[:, h * 128 : (h + 1) * 128], ident)
                nc.vector.tensor_copy(out=outt[:, h, :], in_=ps_tt)
            off = b * NB + j * MT
            nc.sync.dma_start(out=out_d[:, :, off : off + MT], in_=outt)

        for b in range(NBLK):
            if b + 3 <= NQC - 1:
                f1_fetch(b + 3)
            for g in range(NG):
                if g == 5 and b + 2 <= NQC - 1:
                    q_proj(b + 2)
                j, half = g // 2, g % 2
                if b + 1 < NBLK:
                    s_and_exp(b + 1, g)
                if half == 0:
                    accs[(b, j)] = pp.tile(
                        [128, CA], F32, tag="acc", bufs=2, name="acc"
                    )
                acc = accs[(b, j)]
                for t in range(16):
                    u = half * 16 + t
                    eg = es_map[(b, u // GS)]
                    nc.tensor.matmul(
                        acc,
                        lhsT=eg[:, u % GS, j * MT : (j + 1) * MT],
                        rhs=vT_sb[:, u, :],
                        start=(u == 0), stop=(u == NMT - 1),
                    )
                # deferred epilogues, placed to give the DVE chain runway
                # before the PE consumes its results
                if g == 0 and b > 0:
                    flush(b - 1, 3)
                elif g in (2, 4, 6):
                    flush(b, g // 2 - 1)
                if half == 1:
                    norm(b, j)
            for g in range(NG):
                es_map.pop((b, g))
        flush(NBLK - 1, 3)

    nc.finalize()
    return nc


def _bf16(x):
    return np.asarray(np.asarray(x, np.float32), ml_dtypes.bfloat16)


def _prep_core_inputs(inputs, b):
    f1 = _bf16(inputs["feature1"][b].reshape(CH, 128, HW).transpose(1, 0, 2))
    f2 = _bf16(inputs["feature2"][b].reshape(CH, 128, HW).transpose(1, 0, 2))
    f3 = _bf16(inputs["feature3"][b].reshape(CH, 128, HW).transpose(1, 0, 2))
    wqT = inputs["wq"].T.reshape(CH, 128, CQK).transpose(1, 0, 2)
    wkT = inputs["wk"].T.reshape(CH, 128, CQK).transpose(1, 0, 2)
    wq4 = _bf16(np.tile(wqT, (1, 1, 4)))
    wk4 = _bf16(np.tile(wkT, (1, 1, 4)))
    wvT = _bf16(inputs["wv"].T.reshape(CH, 128, C).transpose(1, 0, 2))
    return {
        "f1": np.ascontiguousarray(f1),
        "f2": np.ascontiguousarray(f2),
        "f3": np.ascontiguousarray(f3),
        "wq4": np.ascontiguousarray(wq4),
        "wk4": np.ascontiguousarray(wk4),
        "wvT": np.ascontiguousarray(wvT),
        "bq4": np.ascontiguousarray(np.tile(inputs["bq"], 4).reshape(128, 1)),
        "bvr": np.ascontiguousarray(_bf16(inputs["bv"].reshape(1, C))),
    }


def run_sharded(inputs, trace=False, **kwargs):
    """Shard over batch, run on 8 cores, gather. Returns (output, results)."""
    global _CACHED_NC
    inputs = {k: np.asarray(v, dtype=np.float32) for k, v in inputs.items()}
    if _CACHED_NC is None:
        _CACHED_NC = build_nc()
    nc = _CACHED_NC
    in_maps = [_prep_core_inputs(inputs, b) for b in range(B)]
    results = run_bass_kernel_spmd(
        nc, in_maps, core_ids=list(range(B)), trace=trace, **kwargs
    )
    out = np.stack(
        [
            np.asarray(r["out"]).transpose(1, 0, 2).reshape(C, H, W)
            for r in results.results
        ]
    )
    return out.astype(np.float32), results


def kernel(**inputs) -> np.ndarray:
    out, _ = run_sharded(inputs, trace=False)
    return out


# revision 29
# speedup vs baseline: 1.0677x; 1.0677x over previous
"""Cross-attention kernel for Trainium2 (Bass/Tile), 8-core data-parallel over batch.

Problem (per batch element b, all fp32):
    q = wq @ f1 + bq            # [32, 4096]
    k = wk @ f2 + bk            # [32, 4096]
    v = wv @ f3 + bv            # [256, 4096]
    A = softmax(q^T k, axis=m)  # [4096, 4096]   (n = query pixel, m = key pixel)
    out[c, n] = sum_m v[c, m] * A[n, m]          # [256, 4096]

Kernel strategy (flash-style, no HBM attention slab), v3:
  - One batch element per NeuronCore (B=8, 8 cores).
  - S^T tiles (m on partitions) so exp(S^T) feeds the O matmul as lhsT with
    zero transposes in the attention inner loop.
  - q/k are projected with 4x-replicated weights so q[c,n]/k[c,m] live in all
    four 32-partition groups; the K=32 S^T matmuls are then issued 4 at a time
    to distinct PE row-groups via tile_position (near-4x concurrency).
  - Everything bf16 (features, weights, q/k, exp(S), v^T): halves feature DMA
    (6.3MB/core), gives LDWEIGHTS fast-weight-load, keeps matmuls at
    1 cycle/row.  PSUM accumulation stays fp32.
  - Software pipeline: S^T+exp of block b+1 are emitted interleaved with the
    O-accumulation matmuls of block b, so the Scalar engine's exp (~2us per
    4-tile group, ~126us total) hides under PE work and the PE never idles
    long enough for HAM to re-throttle the clock.
  - A dozen fp32 identity matmuls at t~8us warm the HAM clock gate before the
    first real projection (cold PE runs at 1.2GHz vs 2.4GHz warm).
  - Phase 1 is ordered so the exp(block 0) Scalar-engine chain (the real
    phase-1 critical path) starts as early as possible: f1 chunk 0, then f2
    chunks (k projection + S^T(0,g) + exp trailing each), with f3/v-proj
    interleaved into the exp-paced slack.
  - Softmax denominators come free from a ones-column appended to v^T
    (CA=258 columns: 256 + ones + pad).  bv added at the very end.
"""

import numpy as np
import ml_dtypes
from contextlib import ExitStack

import concourse.bass as bass
import concourse.bacc as bacc
import concourse.tile as tile
from concourse import mybir
from concourse.bass_utils import run_bass_kernel_spmd
from concourse.masks import make_identity

F32 = mybir.dt.float32
BF16 = mybir.dt.bfloat16

B, C, H, W = 8, 256, 64, 64
HW = H * W                     # 4096
CQK = C // 8                   # 32
NB = 512                       # query-pixel block (free dim of S^T matmuls)
NBLK = HW // NB                # 8
MT = 128                       # key-pixel tile (partition dim of S^T)
NMT = HW // MT                 # 32
GS = 4                         # S^T matmuls packed per PE row-group volley
NG = NMT // GS                 # 8 packed groups per block
CH = C // 128                  # 2 channel halves
QCH = 512                      # projection chunk
NQC = HW // QCH                # 8
CA = C + 2                     # v_aug columns (ones + pad)

_CACHED_NC = None


def build_nc():
    nc = bacc.Bacc("TRN2")

    f1_d = nc.dram_tensor("f1", [128, CH, HW], BF16, kind="ExternalInput")
    f2_d = nc.dram_tensor("f2", [128, CH, HW], BF16, kind="ExternalInput")
    f3_d = nc.dram_tensor("f3", [128, CH, HW], BF16, kind="ExternalInput")
    wq4_d = nc.dram_tensor("wq4", [128, CH, 128], BF16, kind="ExternalInput")
    wk4_d = nc.dram_tensor("wk4", [128, CH, 128], BF16, kind="ExternalInput")
    wvT_d = nc.dram_tensor("wvT", [128, CH, C], BF16, kind="ExternalInput")
    bq4_d = nc.dram_tensor("bq4", [128, 1], F32, kind="ExternalInput")
    bvr_d = nc.dram_tensor("bvr", [1, C], BF16, kind="ExternalInput")
    out_d = nc.dram_tensor("out", [128, CH, HW], BF16, kind="ExternalOutput")

    with tile.TileContext(nc) as tc, ExitStack() as octx:
        const = octx.enter_context(tc.tile_pool(name="const", bufs=1))
        persist = octx.enter_context(tc.tile_pool(name="persist", bufs=1))
        fpool = octx.enter_context(tc.tile_pool(name="fpool", bufs=3))
        espool = octx.enter_context(tc.tile_pool(name="es", bufs=16))
        pp = octx.enter_context(tc.tile_pool(name="pp", bufs=1, space="PSUM"))
        opool = octx.enter_context(tc.tile_pool(name="outp", bufs=4))
        rpool = octx.enter_context(tc.tile_pool(name="rp", bufs=4))

        ident = const.tile([128, 128], F32)
        make_identity(nc, ident)
        wq4_sb = const.tile([128, CH, 128], BF16)
        wk4_sb = const.tile([128, CH, 128], BF16)
        wv_sb = const.tile([128, CH, C], BF16)
        bq4_sb = const.tile([128, 1], F32)
        bvr_sb = const.tile([1, C], BF16)
        ones1_sb = const.tile([1, 128], BF16)
        nc.vector.memset(ones1_sb, 1.0)
        # DMA issue order matters: the Sync queue issues one dma_start every
        # ~0.6us, so the tensors needed by the first matmuls go first (wv/bvr
        # are issued inside the f2 loop, after f2 chunk 0).
        nc.sync.dma_start(out=wk4_sb, in_=wk4_d[:])
        nc.sync.dma_start(out=wq4_sb, in_=wq4_d[:])

        # warm the HAM clock gate with ~5us of throwaway fp32 matmuls while
        # the first DMAs are in flight (cold PE = half clock; the activity
        # monitor needs ~3.4us of sustained matmul work inside one of its
        # free-running windows to unthrottle, and the warmup must also bridge
        # the PE to the arrival of the first feature chunk so no >3.4us idle
        # gap re-throttles it)
        for _ in range(16):
            ps_w = pp.tile([128, MT], F32, tag="tt", bufs=2, name="ps_w")
            nc.tensor.matmul(ps_w, lhsT=ident, rhs=ident, start=True, stop=True)

        # persistent products of phase 1 (q/k replicated across the 4
        # partition groups by construction of the replicated weights)
        q_sb = persist.tile([128, HW], BF16)
        k_sb = persist.tile([128, HW], BF16)
        vT_sb = persist.tile([128, NMT, CA], BF16)  # [128, 32, 258]
        ones_sb = const.tile([128, NMT, 2], F32)
        nc.vector.memset(ones_sb[:, :, 0:1], 1.0)
        nc.vector.memset(ones_sb[:, :, 1:2], 0.0)
        nc.vector.tensor_copy(out=vT_sb[:, :, C:CA], in_=ones_sb)

        es_map = {}

        def s_and_exp(b, g):
            """Emit 4 row-group-packed S^T matmuls (m-tiles 4g..4g+3 of query
            block b) + one exp activation over the 4-bank PSUM group."""
            nsl = slice(b * NB, (b + 1) * NB)
            ps_s = pp.tile([128, GS, NB], F32, tag="s", bufs=1, name="ps_s")
            for i in range(GS):
                u = g * GS + i
                nc.tensor.matmul(
                    ps_s[:, i, :],
                    lhsT=k_sb[32 * i : 32 * i + 32, u * MT : (u + 1) * MT],
                    rhs=q_sb[32 * i : 32 * i + 32, nsl],
                    start=True, stop=True,
                    tile_position=(32 * i, 0),
                )
            es_g = espool.tile([128, GS, NB], BF16, tag="es", bufs=16, name="es_g")
            nc.scalar.activation(
                out=es_g, in_=ps_s, func=mybir.ActivationFunctionType.Exp
            )
            es_map[(b, g)] = es_g

        f1_tiles = {}

        def f1_fetch(c):
            fch = fpool.tile([128, CH, QCH], BF16, tag="f1", bufs=3, name="f1ch")
            nc.sync.dma_start(out=fch, in_=f1_d[:, :, c * QCH : (c + 1) * QCH])
            f1_tiles[c] = fch

        def q_proj(c):
            sl = slice(c * QCH, (c + 1) * QCH)
            ps_q = pp.tile([128, QCH], F32, tag="tt", bufs=2, name="ps_q")
            nc.tensor.matmul(
                ps_q, lhsT=wq4_sb[:, 0, :], rhs=f1_tiles[c][:, 0, :],
                start=True, stop=False,
            )
            nc.tensor.matmul(
                ps_q, lhsT=wq4_sb[:, 1, :], rhs=f1_tiles[c][:, 1, :],
                start=False, stop=True,
            )
            nc.vector.tensor_scalar_add(out=q_sb[:, sl], in0=ps_q, scalar1=bq4_sb)
            del f1_tiles[c]

        def v_proj(j):
            """v^T tiles for chunk j, with bv folded in via a rank-1 matmul
            (v' = v + bv makes the final output (sum es*v')/denom = out
            directly, since softmax rows sum to 1)."""
            fch3 = fpool.tile([128, CH, QCH], BF16, tag="f3", bufs=3, name="f3ch")
            nc.sync.dma_start(out=fch3, in_=f3_d[:, :, j * QCH : (j + 1) * QCH])
            for i2 in range(2):
                # acc tag: idle during phase 1, so v-projection PSUM lives
                # there instead of contending with the k/q chain on "tt"
                ps_v = pp.tile([128, 2, C], F32, tag="acc", bufs=2, name="ps_v")
                for i in range(2):
                    isl = slice((i2 * 2 + i) * MT, (i2 * 2 + i + 1) * MT)
                    nc.tensor.matmul(
                        ps_v[:, i, :], lhsT=fch3[:, 0, isl], rhs=wv_sb[:, 0, :],
                        start=True, stop=False,
                    )
                    nc.tensor.matmul(
                        ps_v[:, i, :], lhsT=fch3[:, 1, isl], rhs=wv_sb[:, 1, :],
                        start=False, stop=False,
                    )
                    nc.tensor.matmul(
                        ps_v[:, i, :], lhsT=ones1_sb, rhs=bvr_sb,
                        start=False, stop=True,
                    )
                u = j * 4 + i2 * 2
                nc.vector.tensor_copy(out=vT_sb[:, u : u + 2, 0:C], in_=ps_v)

        # ---- phase 1 ----
        # f1 chunk 0 (for q chunk 0), then f2 chunks with k-projection +
        # S^T(0,g) + exp trailing each (k chunk g holds exactly m-tiles
        # 4g..4g+3 = S-group g); f3/v-projection fills the exp-paced slack.
        f1_fetch(0)
        nc.sync.dma_start(out=bq4_sb, in_=bq4_d[:])
        q_proj(0)
        for g in range(NQC):
            sl = slice(g * QCH, (g + 1) * QCH)
            fch2 = fpool.tile([128, CH, QCH], BF16, tag="f2", bufs=3, name="f2ch")
            nc.sync.dma_start(out=fch2, in_=f2_d[:, :, sl])
            if g == 0:
                nc.sync.dma_start(out=wv_sb, in_=wvT_d[:])
                nc.sync.dma_start(out=bvr_sb, in_=bvr_d[:])
            ps_k = pp.tile([128, QCH], F32, tag="tt", bufs=2, name="ps_k")
            nc.tensor.matmul(
                ps_k, lhsT=wk4_sb[:, 0, :], rhs=fch2[:, 0, :],
                start=True, stop=False,
            )
            nc.tensor.matmul(
                ps_k, lhsT=wk4_sb[:, 1, :], rhs=fch2[:, 1, :],
                start=False, stop=True,
            )
            # no k bias: softmax over m is invariant to the bk term, which
            # only adds n-dependent and constant offsets to q^T k
            nc.vector.tensor_copy(out=k_sb[:, sl], in_=ps_k)
            s_and_exp(0, g)
            if g >= 1:
                v_proj(g - 1)
            # dependency-free filler matmuls: phase 1 is exp-paced and the PE
            # would otherwise idle ~50% of each ring step, which can trip the
            # HAM activity monitor back to half clock mid-phase
            for _ in range(3):
                ps_w = pp.tile([128, MT], F32, tag="tt", bufs=2, name="ps_w")
                nc.tensor.matmul(ps_w, lhsT=ident, rhs=ident, start=True, stop=True)
        v_proj(NQC - 1)

        f1_fetch(1)
        f1_fetch(2)
        q_proj(1)

        # ---- phase 2: pipelined attention ----
        accs = {}
        onrms = {}

        def norm(b, j):
            """DVE part of the epilogue: 1/denominator, normalize (bv is
            already folded into v^T, so this is the final value)."""
            acc = accs.pop((b, j))
            rcp = rpool.tile([128, 1], F32, tag="r", name="rcp")
            nc.vector.reciprocal(rcp, acc[:, C : C + 1])
            onrm = rpool.tile([128, C], F32, tag="onrm", name="onrm")
            nc.vector.tensor_scalar_mul(onrm, acc[:, 0:C], rcp)
            onrms[(b, j)] = onrm

        def flush(b, j):
            """PE transposes [n, c] -> [c, n] + bf16 store (bv already folded
            into v^T, so no bias add here)."""
            onrm = onrms.pop((b, j))
            outt = opool.tile([128, CH, MT], BF16, tag="out", name="outt")
            for h in range(CH):
                ps_tt = pp.tile([128, MT], F32, tag="tt", bufs=2, name="ps_tt")
                nc.tensor.transpose(ps_tt, onrm[:, h * 128 : (h + 1) * 128], ident)
                nc.vector.tensor_copy(out=outt[:, h, :], in_=ps_tt)
            off = b * NB + j * MT
            nc.sync.dma_start(out=out_d[:, :, off : off + MT], in_=outt)

        for b in range(NBLK):
            if b + 3 <= NQC - 1:
                f1_fetch(b + 3)
            for g in range(NG):
                if g == 5 and b + 2 <= NQC - 1:
                    q_proj(b + 2)
                j, half = g // 2, g % 2
                if b + 1 < NBLK:
                    s_and_exp(b + 1, g)
                if half == 0:
                    accs[(b, j)] = pp.tile(
                        [128, CA], F32, tag="acc", bufs=2, name="acc"
                    )
                acc = accs[(b, j)]
                for t in range(16):
                    u = half * 16 + t
                    eg = es_map[(b, u // GS)]
                    nc.tensor.matmul(
                        acc,
                        lhsT=eg[:, u % GS, j * MT : (j + 1) * MT],
                        rhs=vT_sb[:, u, :],
                        start=(u == 0), stop=(u == NMT - 1),
                    )
                # deferred epilogues, placed to give the DVE chain runway
                # before the PE consumes its results
                if g == 0 and b > 0:
                    flush(b - 1, 3)
                elif g in (2, 4, 6):
                    flush(b, g // 2 - 1)
                if half == 1:
                    norm(b, j)
            for g in range(NG):
                es_map.pop((b, g))
        flush(NBLK - 1, 3)

    nc.finalize()
    return nc


def _bf16(x):
    return np.asarray(np.asarray(x, np.float32), ml_dtypes.bfloat16)


def _prep_core_inputs(inputs, b):
    f1 = _bf16(inputs["feature1"][b].reshape(CH, 128, HW).transpose(1, 0, 2))
    f2 = _bf16(inputs["feature2"][b].reshape(CH, 128, HW).transpose(1, 0, 2))
    f3 = _bf16(inputs["feature3"][b].reshape(CH, 128, HW).transpose(1, 0, 2))
    wqT = inputs["wq"].T.reshape(CH, 128, CQK).transpose(1, 0, 2)
    wkT = inputs["wk"].T.reshape(CH, 128, CQK).transpose(1, 0, 2)
    wq4 = _bf16(np.tile(wqT, (1, 1, 4)))
    wk4 = _bf16(np.tile(wkT, (1, 1, 4)))
    wvT = _bf16(inputs["wv"].T.reshape(CH, 128, C).transpose(1, 0, 2))
    return {
        "f1": np.ascontiguousarray(f1),
        "f2": np.ascontiguousarray(f2),
        "f3": np.ascontiguousarray(f3),
        "wq4": np.ascontiguousarray(wq4),
        "wk4": np.ascontiguousarray(wk4),
        "wvT": np.ascontiguousarray(wvT),
        "bq4": np.ascontiguousarray(np.tile(inputs["bq"], 4).reshape(128, 1)),
        "bvr": np.ascontiguousarray(_bf16(inputs["bv"].reshape(1, C))),
    }


def run_sharded(inputs, trace=False, **kwargs):
    """Shard over batch, run on 8 cores, gather. Returns (output, results)."""
    global _CACHED_NC
    inputs = {k: np.asarray(v, dtype=np.float32) for k, v in inputs.items()}
    if _CACHED_NC is None:
        _CACHED_NC = build_nc()
    nc = _CACHED_NC
    in_maps = [_prep_core_inputs(inputs, b) for b in range(B)]
    results = run_bass_kernel_spmd(
        nc, in_maps, core_ids=list(range(B)), trace=trace, **kwargs
    )
    out = np.stack(
        [
            np.asarray(r["out"]).transpose(1, 0, 2).reshape(C, H, W)
            for r in results.results
        ]
    )
    return out.astype(np.float32), results


def kernel(**inputs) -> np.ndarray:
    out, _ = run_sharded(inputs, trace=False)
    return out


# revision 30
# speedup vs baseline: 1.0829x; 1.0143x over previous
"""Cross-attention kernel for Trainium2 (Bass/Tile), 8-core data-parallel over batch.

Problem (per batch element b, all fp32):
    q = wq @ f1 + bq            # [32, 4096]
    k = wk @ f2 + bk            # [32, 4096]
    v = wv @ f3 + bv            # [256, 4096]
    A = softmax(q^T k, axis=m)  # [4096, 4096]   (n = query pixel, m = key pixel)
    out[c, n] = sum_m v[c, m] * A[n, m]          # [256, 4096]

Kernel strategy (flash-style, no HBM attention slab), v3:
  - One batch element per NeuronCore (B=8, 8 cores).
  - S^T tiles (m on partitions) so exp(S^T) feeds the O matmul as lhsT with
    zero transposes in the attention inner loop.
  - q/k are projected with 4x-replicated weights so q[c,n]/k[c,m] live in all
    four 32-partition groups; the K=32 S^T matmuls are then issued 4 at a time
    to distinct PE row-groups via tile_position (near-4x concurrency).
  - Everything bf16 (features, weights, q/k, exp(S), v^T): halves feature DMA
    (6.3MB/core), gives LDWEIGHTS fast-weight-load, keeps matmuls at
    1 cycle/row.  PSUM accumulation stays fp32.
  - Software pipeline: S^T+exp of block b+1 are emitted interleaved with the
    O-accumulation matmuls of block b, so the Scalar engine's exp (~2us per
    4-tile group, ~126us total) hides under PE work and the PE never idles
    long enough for HAM to re-throttle the clock.
  - A dozen fp32 identity matmuls at t~8us warm the HAM clock gate before the
    first real projection (cold PE runs at 1.2GHz vs 2.4GHz warm).
  - Phase 1 is ordered so the exp(block 0) Scalar-engine chain (the real
    phase-1 critical path) starts as early as possible: f1 chunk 0, then f2
    chunks (k projection + S^T(0,g) + exp trailing each), with f3/v-proj
    interleaved into the exp-paced slack.
  - Softmax denominators come free from a ones-column appended to v^T
    (CA=258 columns: 256 + ones + pad).  bv added at the very end.
"""

import numpy as np
import ml_dtypes
from contextlib import ExitStack

import concourse.bass as bass
import concourse.bacc as bacc
import concourse.tile as tile
from concourse import mybir
from concourse.bass_utils import run_bass_kernel_spmd
from concourse.masks import make_identity

F32 = mybir.dt.float32
BF16 = mybir.dt.bfloat16

B, C, H, W = 8, 256, 64, 64
HW = H * W                     # 4096
CQK = C // 8                   # 32
NB = 512                       # query-pixel block (free dim of S^T matmuls)
NBLK = HW // NB                # 8
MT = 128                       # key-pixel tile (partition dim of S^T)
NMT = HW // MT                 # 32
GS = 4                         # S^T matmuls packed per PE row-group volley
NG = NMT // GS                 # 8 packed groups per block
CH = C // 128                  # 2 channel halves
QCH = 512                      # projection chunk
NQC = HW // QCH                # 8
CA = C + 2                     # v_aug columns (ones + pad)

_CACHED_NC = None


def build_nc():
    nc = bacc.Bacc("TRN2")

    f1_d = nc.dram_tensor("f1", [128, CH, HW], BF16, kind="ExternalInput")
    f2_d = nc.dram_tensor("f2", [128, CH, HW], BF16, kind="ExternalInput")
    f3_d = nc.dram_tensor("f3", [128, CH, HW], BF16, kind="ExternalInput")
    wq4_d = nc.dram_tensor("wq4", [128, CH, 128], BF16, kind="ExternalInput")
    wk4_d = nc.dram_tensor("wk4", [128, CH, 128], BF16, kind="ExternalInput")
    wvT_d = nc.dram_tensor("wvT", [128, CH, C], BF16, kind="ExternalInput")
    bq4_d = nc.dram_tensor("bq4", [128, 1], F32, kind="ExternalInput")
    bvr_d = nc.dram_tensor("bvr", [1, C], BF16, kind="ExternalInput")
    out_d = nc.dram_tensor("out", [128, CH, HW], BF16, kind="ExternalOutput")

    with tile.TileContext(nc) as tc, ExitStack() as octx:
        const = octx.enter_context(tc.tile_pool(name="const", bufs=1))
        persist = octx.enter_context(tc.tile_pool(name="persist", bufs=1))
        fpool = octx.enter_context(tc.tile_pool(name="fpool", bufs=3))
        espool = octx.enter_context(tc.tile_pool(name="es", bufs=16))
        pp = octx.enter_context(tc.tile_pool(name="pp", bufs=1, space="PSUM"))
        opool = octx.enter_context(tc.tile_pool(name="outp", bufs=4))
        rpool = octx.enter_context(tc.tile_pool(name="rp", bufs=4))

        ident = const.tile([128, 128], F32)
        make_identity(nc, ident)
        wq4_sb = const.tile([128, CH, 128], BF16)
        wk4_sb = const.tile([128, CH, 128], BF16)
        wv_sb = const.tile([128, CH, C], BF16)
        bq4_sb = const.tile([128, 1], F32)
        bvr_sb = const.tile([1, C], BF16)
        ones1_sb = const.tile([1, 128], BF16)
        nc.vector.memset(ones1_sb, 1.0)
        # DMA issue order matters: the Sync queue issues one dma_start every
        # ~0.6us, so the tensors needed by the first matmuls go first (wv/bvr
        # are issued inside the f2 loop, after f2 chunk 0).
        nc.sync.dma_start(out=wk4_sb, in_=wk4_d[:])
        nc.sync.dma_start(out=wq4_sb, in_=wq4_d[:])

        # warm the HAM clock gate with ~5us of throwaway fp32 matmuls while
        # the first DMAs are in flight (cold PE = half clock; the activity
        # monitor needs ~3.4us of sustained matmul work inside one of its
        # free-running windows to unthrottle, and the warmup must also bridge
        # the PE to the arrival of the first feature chunk so no >3.4us idle
        # gap re-throttles it)
        for _ in range(14):
            ps_w = pp.tile([128, MT], F32, tag="tt", bufs=2, name="ps_w")
            nc.tensor.matmul(ps_w, lhsT=ident, rhs=ident, start=True, stop=True)

        # persistent products of phase 1 (q/k replicated across the 4
        # partition groups by construction of the replicated weights)
        q_sb = persist.tile([128, HW], BF16)
        k_sb = persist.tile([128, HW], BF16)
        vT_sb = persist.tile([128, NMT, CA], BF16)  # [128, 32, 258]
        ones_sb = const.tile([128, NMT, 2], F32)
        nc.vector.memset(ones_sb[:, :, 0:1], 1.0)
        nc.vector.memset(ones_sb[:, :, 1:2], 0.0)
        nc.vector.tensor_copy(out=vT_sb[:, :, C:CA], in_=ones_sb)

        es_map = {}

        def s_and_exp(b, g):
            """Emit 4 row-group-packed S^T matmuls (m-tiles 4g..4g+3 of query
            block b) + one exp activation over the 4-bank PSUM group."""
            nsl = slice(b * NB, (b + 1) * NB)
            ps_s = pp.tile([128, GS, NB], F32, tag="s", bufs=1, name="ps_s")
            for i in range(GS):
                u = g * GS + i
                nc.tensor.matmul(
                    ps_s[:, i, :],
                    lhsT=k_sb[32 * i : 32 * i + 32, u * MT : (u + 1) * MT],
                    rhs=q_sb[32 * i : 32 * i + 32, nsl],
                    start=True, stop=True,
                    tile_position=(32 * i, 0),
                )
            es_g = espool.tile([128, GS, NB], BF16, tag="es", bufs=16, name="es_g")
            nc.scalar.activation(
                out=es_g, in_=ps_s, func=mybir.ActivationFunctionType.Exp
            )
            es_map[(b, g)] = es_g

        f1_tiles = {}

        def f1_fetch(c):
            fch = fpool.tile([128, CH, QCH], BF16, tag="f1", bufs=3, name="f1ch")
            nc.sync.dma_start(out=fch, in_=f1_d[:, :, c * QCH : (c + 1) * QCH])
            f1_tiles[c] = fch

        def q_proj(c):
            sl = slice(c * QCH, (c + 1) * QCH)
            ps_q = pp.tile([128, QCH], F32, tag="tt", bufs=2, name="ps_q")
            nc.tensor.matmul(
                ps_q, lhsT=wq4_sb[:, 0, :], rhs=f1_tiles[c][:, 0, :],
                start=True, stop=False,
            )
            nc.tensor.matmul(
                ps_q, lhsT=wq4_sb[:, 1, :], rhs=f1_tiles[c][:, 1, :],
                start=False, stop=True,
            )
            nc.vector.tensor_scalar_add(out=q_sb[:, sl], in0=ps_q, scalar1=bq4_sb)
            del f1_tiles[c]

        def v_proj(j):
            """v^T tiles for chunk j, with bv folded in via a rank-1 matmul
            (v' = v + bv makes the final output (sum es*v')/denom = out
            directly, since softmax rows sum to 1)."""
            fch3 = fpool.tile([128, CH, QCH], BF16, tag="f3", bufs=3, name="f3ch")
            nc.sync.dma_start(out=fch3, in_=f3_d[:, :, j * QCH : (j + 1) * QCH])
            for i2 in range(2):
                # acc tag: idle during phase 1, so v-projection PSUM lives
                # there instead of contending with the k/q chain on "tt"
                ps_v = pp.tile([128, 2, C], F32, tag="acc", bufs=2, name="ps_v")
                for i in range(2):
                    isl = slice((i2 * 2 + i) * MT, (i2 * 2 + i + 1) * MT)
                    nc.tensor.matmul(
                        ps_v[:, i, :], lhsT=fch3[:, 0, isl], rhs=wv_sb[:, 0, :],
                        start=True, stop=False,
                    )
                    nc.tensor.matmul(
                        ps_v[:, i, :], lhsT=fch3[:, 1, isl], rhs=wv_sb[:, 1, :],
                        start=False, stop=False,
                    )
                    nc.tensor.matmul(
                        ps_v[:, i, :], lhsT=ones1_sb, rhs=bvr_sb,
                        start=False, stop=True,
                    )
                u = j * 4 + i2 * 2
                nc.vector.tensor_copy(out=vT_sb[:, u : u + 2, 0:C], in_=ps_v)

        # ---- phase 1 ----
        # f1 chunk 0 (for q chunk 0), then f2 chunks with k-projection +
        # S^T(0,g) + exp trailing each (k chunk g holds exactly m-tiles
        # 4g..4g+3 = S-group g); f3/v-projection fills the exp-paced slack.
        f1_fetch(0)
        nc.sync.dma_start(out=bq4_sb, in_=bq4_d[:])
        q_proj(0)
        for g in range(NQC):
            sl = slice(g * QCH, (g + 1) * QCH)
            fch2 = fpool.tile([128, CH, QCH], BF16, tag="f2", bufs=3, name="f2ch")
            nc.sync.dma_start(out=fch2, in_=f2_d[:, :, sl])
            if g == 0:
                nc.sync.dma_start(out=wv_sb, in_=wvT_d[:])
                nc.sync.dma_start(out=bvr_sb, in_=bvr_d[:])
            ps_k = pp.tile([128, QCH], F32, tag="tt", bufs=2, name="ps_k")
            nc.tensor.matmul(
                ps_k, lhsT=wk4_sb[:, 0, :], rhs=fch2[:, 0, :],
                start=True, stop=False,
            )
            nc.tensor.matmul(
                ps_k, lhsT=wk4_sb[:, 1, :], rhs=fch2[:, 1, :],
                start=False, stop=True,
            )
            # no k bias: softmax over m is invariant to the bk term, which
            # only adds n-dependent and constant offsets to q^T k
            nc.vector.tensor_copy(out=k_sb[:, sl], in_=ps_k)
            s_and_exp(0, g)
            if g >= 1:
                v_proj(g - 1)
            # dependency-free filler matmuls: phase 1 is exp-paced and the PE
            # would otherwise idle ~50% of each ring step, which can trip the
            # HAM activity monitor back to half clock mid-phase
            for _ in range(2):
                ps_w = pp.tile([128, MT], F32, tag="tt", bufs=2, name="ps_w")
                nc.tensor.matmul(ps_w, lhsT=ident, rhs=ident, start=True, stop=True)
        v_proj(NQC - 1)

        f1_fetch(1)
        f1_fetch(2)
        q_proj(1)

        # ---- phase 2: pipelined attention ----
        accs = {}
        onrms = {}

        def norm(b, j):
            """DVE part of the epilogue: 1/denominator, normalize (bv is
            already folded into v^T, so this is the final value)."""
            acc = accs.pop((b, j))
            rcp = rpool.tile([128, 1], F32, tag="r", name="rcp")
            nc.vector.reciprocal(rcp, acc[:, C : C + 1])
            onrm = rpool.tile([128, C], F32, tag="onrm", name="onrm")
            nc.vector.tensor_scalar_mul(onrm, acc[:, 0:C], rcp)
            onrms[(b, j)] = onrm

        def flush(b, j):
            """PE transposes [n, c] -> [c, n] + bf16 store (bv already folded
            into v^T, so no bias add here)."""
            onrm = onrms.pop((b, j))
            outt = opool.tile([128, CH, MT], BF16, tag="out", name="outt")
            for h in range(CH):
                ps_tt = pp.tile([128, MT], F32, tag="tt", bufs=2, name="ps_tt")
                nc.tensor.transpose(ps_tt, onrm[:, h * 128 : (h + 1) * 128], ident)
                nc.vector.tensor_copy(out=outt[:, h, :], in_=ps_tt)
            off = b * NB + j * MT
            nc.sync.dma_start(out=out_d[:, :, off : off + MT], in_=outt)

        for b in range(NBLK):
            if b + 3 <= NQC - 1:
                f1_fetch(b + 3)
            for g in range(NG):
                if g == 5 and b + 2 <= NQC - 1:
                    q_proj(b + 2)
                j, half = g // 2, g % 2
                if b + 1 < NBLK:
                    s_and_exp(b + 1, g)
                if half == 0:
                    accs[(b, j)] = pp.tile(
                        [128, CA], F32, tag="acc", bufs=2, name="acc"
                    )
                acc = accs[(b, j)]
                for t in range(16):
                    u = half * 16 + t
                    eg = es_map[(b, u // GS)]
                    nc.tensor.matmul(
                        acc,
                        lhsT=eg[:, u % GS, j * MT : (j + 1) * MT],
                        rhs=vT_sb[:, u, :],
                        start=(u == 0), stop=(u == NMT - 1),
                    )
                # deferred epilogues, placed to give the DVE chain runway
                # before the PE consumes its results
                if g == 0 and b > 0:
                    flush(b - 1, 3)
                elif g in (2, 4, 6):
                    flush(b, g // 2 - 1)
                if half == 1:
                    norm(b, j)
            for g in range(NG):
                es_map.pop((b, g))
        flush(NBLK - 1, 3)

    nc.finalize()
    return nc


def _bf16(x):
    return np.asarray(np.asarray(x, np.float32), ml_dtypes.bfloat16)


def _prep_core_inputs(inputs, b):
    f1 = _bf16(inputs["feature1"][b].reshape(CH, 128, HW).transpose(1, 0, 2))
    f2 = _bf16(inputs["feature2"][b].reshape(CH, 128, HW).transpose(1, 0, 2))
    f3 = _bf16(inputs["feature3"][b].reshape(CH, 128, HW).transpose(1, 0, 2))
    wqT = inputs["wq"].T.reshape(CH, 128, CQK).transpose(1, 0, 2)
    wkT = inputs["wk"].T.reshape(CH, 128, CQK).transpose(1, 0, 2)
    wq4 = _bf16(np.tile(wqT, (1, 1, 4)))
    wk4 = _bf16(np.tile(wkT, (1, 1, 4)))
    wvT = _bf16(inputs["wv"].T.reshape(CH, 128, C).transpose(1, 0, 2))
    return {
        "f1": np.ascontiguousarray(f1),
        "f2": np.ascontiguousarray(f2),
        "f3": np.ascontiguousarray(f3),
        "wq4": np.ascontiguousarray(wq4),
        "wk4": np.ascontiguousarray(wk4),
        "wvT": np.ascontiguousarray(wvT),
        "bq4": np.ascontiguousarray(np.tile(inputs["bq"], 4).reshape(128, 1)),
        "bvr": np.ascontiguousarray(_bf16(inputs["bv"].reshape(1, C))),
    }


def run_sharded(inputs, trace=False, **kwargs):
    """Shard over batch, run on 8 cores, gather. Returns (output, results)."""
    global _CACHED_NC
    inputs = {k: np.asarray(v, dtype=np.float32) for k, v in inputs.items()}
    if _CACHED_NC is None:
        _CACHED_NC = build_nc()
    nc = _CACHED_NC
    in_maps = [_prep_core_inputs(inputs, b) for b in range(B)]
    results = run_bass_kernel_spmd(
        nc, in_maps, core_ids=list(range(B)), trace=trace, **kwargs
    )
    out = np.stack(
        [
            np.asarray(r["out"]).transpose(1, 0, 2).reshape(C, H, W)
            for r in results.results
        ]
    )
    return out.astype(np.float32), results


def kernel(**inputs) -> np.ndarray:
    out, _ = run_sharded(inputs, trace=False)
    return out
